# revision 18
# baseline (speedup 1.0000x reference)
"""MultiHeadAttention kernel for Trainium2, 8 NeuronCores.

Problem shapes (hardcoded): B=4, C=256, N=M=4096, H=4 heads, D=64 head dim.
reference: Q/K/V = 1x1-conv projections, scores = Q^T K / sqrt(D) per head,
softmax over source dim, out = attn @ V^T, merge projection.

Sharding: 8 cores = (batch b, query-half nh). Each core computes the full
output rows for its (b, n-half): K/V projections are recomputed per n-half
(5% redundant FLOPs) which keeps every core's output slice disjoint — the
host just concatenates, no reductions.

Per-core dataflow (bf16 matmul operands, fp32 PSUM accumulation):
  K  = WkT.T @ src            (c_out on partitions, m free)    [PE]
  Q  = WqT.T @ q              (c_out on partitions, n free)    [PE]
  VT = src.T @ WvT            (m on partitions, c_out free)    [PE]
  per (head h, n-tile of 1024):
    scoresT[m,n] = K_h^T Q_h  m-chunks of 128 partitions       [PE]
    probsT = exp(scoresT/8)   fp32 PSUM -> bf16 SBUF (no max
                              subtraction: scores ~ N(0,1))    [ACT]
    outT(n,65) += probsT_chunk.T @ [VT_h | ones]  -> row 64 is the
                              softmax denominator              [PE]
    r = 1/denoms (approx-NR recip); partition-broadcast via a
    DRAM bounce; attn_h = outT * r                             [DVE+DMA]
  y = WmT.T @ attn + bm       contract heads, K=64 each        [PE]

The kernel is ACT(exp)-bound: 256 exp ops x (128,1024) ~ 285us/core is
the floor; scores/outT matmuls software-pipeline (LAG=2) under it, and
fp32 matmuls are avoided (fp32_mode=LOW_HIGH is 2 passes through PE).
Measured: ~344us HW exec, rel err ~4.7e-3 vs fp32 reference.

Hardware landmines hit during bring-up (kept working around):
  - gpsimd.partition_broadcast reads the wrong partition for inputs not
    based at partition 0, and heavy gpsimd SBUF traffic locks the
    DVE-shared port;
  - DMA with partition-shifted or partition-step-0 SBUF APs hangs the
    device (DRAM-side broadcast APs are fine);
  - DVE ops are partition-locked (out/in must share the partition base),
    though a plain reciprocal/copy CAN shift base; the custom-DVE
    reciprocal_approx ops cannot.
"""

import os

import numpy as np

N_CORES = 8
B, C = 4, 256
N = M = 4096
H, D = 4, 64
NN = N // 2          # query positions per core
P = 128
NT = NN // 1024      # n-tiles per core (2)
MC = M // P          # m-chunks (32)

_STATE: dict = {}


def _build():
    from contextlib import ExitStack

    import concourse.bass as bass
    import concourse.mybir as mybir
    import concourse.tile as tile
    from concourse import bacc

    f32 = mybir.dt.float32
    bf16 = mybir.dt.bfloat16
    Exp = mybir.ActivationFunctionType.Exp
    add = mybir.AluOpType.add
    mult = mybir.AluOpType.mult

    nc = bacc.Bacc(
        "TRN2",
        target_bir_lowering=False,
        debug=False,
        enable_asserts=False,
        num_devices=N_CORES,
    )

    q_d = nc.dram_tensor("q", (C, NN), bf16, kind="ExternalInput").ap()
    src_d = nc.dram_tensor("src", (C, M), bf16, kind="ExternalInput").ap()
    wqT_d = nc.dram_tensor("wqT", (C, C), bf16, kind="ExternalInput").ap()
    wkT_d = nc.dram_tensor("wkT", (C, C), bf16, kind="ExternalInput").ap()
    wvT_d = nc.dram_tensor("wvT", (C, C), bf16, kind="ExternalInput").ap()
    wmT_d = nc.dram_tensor("wmT", (C, C), bf16, kind="ExternalInput").ap()
    bq_d = nc.dram_tensor("bq", (C,), f32, kind="ExternalInput").ap()
    bk_d = nc.dram_tensor("bk", (C,), f32, kind="ExternalInput").ap()
    bv_d = nc.dram_tensor("bv", (C,), f32, kind="ExternalInput").ap()
    bm_d = nc.dram_tensor("bm", (C,), f32, kind="ExternalInput").ap()
    y_d = nc.dram_tensor("y", (C, NN), f32, kind="ExternalOutput").ap()

    q_r = q_d.rearrange("(a p) n -> a p n", p=P)
    src_r = src_d.rearrange("(a p) n -> a p n", p=P)
    y_r = y_d.rearrange("(a p) n -> a p n", p=P)

    def chunks(w):
        return w.rearrange("(a p) n -> a p n", p=P)

    with tile.TileContext(nc) as tc, ExitStack() as ctx:
        singles = ctx.enter_context(tc.tile_pool(name="singles", bufs=1))
        # PSUM pools: scores 2 banks x2, outT 2 banks x2 = 8 banks total.
        # Projections borrow the scores slots (they run strictly before
        # attention); the merge borrows the outT slots (strictly after).
        spool = ctx.enter_context(tc.tile_pool(name="scores", bufs=2, space="PSUM"))
        opool = ctx.enter_context(tc.tile_pool(name="outps", bufs=2, space="PSUM"))
        probs_p = ctx.enter_context(tc.tile_pool(name="probs", bufs=6))
        small_p = ctx.enter_context(tc.tile_pool(name="small", bufs=4))
        dram_p = ctx.enter_context(tc.tile_pool(name="dram", bufs=2, space="DRAM"))
        ostage = ctx.enter_context(tc.tile_pool(name="ostage", bufs=3))

        # ---- weights / biases ----
        wqt, wkt, wvt = [], [], []
        for ci in range(2):
            for lst, d, nm in ((wqt, wqT_d, "wq"), (wkt, wkT_d, "wk"),
                               (wvt, wvT_d, "wv")):
                t = singles.tile([P, C], bf16, tag=f"{nm}{ci}", name=f"{nm}{ci}")
                nc.gpsimd.dma_start(out=t[:], in_=chunks(d)[ci])
                lst.append(t)
        wm_h = []
        for h in range(H):
            t = singles.tile([D, C], bf16, tag=f"wm{h}", name=f"wm{h}")
            nc.gpsimd.dma_start(out=t[:], in_=wmT_d[h * D:(h + 1) * D, :])
            wm_h.append(t)
        bq_t, bk_t, bm_t = [], [], []
        for ci in range(2):
            for lst, d, nm in ((bq_t, bq_d, "bq"), (bk_t, bk_d, "bk"),
                               (bm_t, bm_d, "bm")):
                t = singles.tile([P, 1], f32, tag=f"{nm}{ci}", name=f"b{nm}{ci}")
                nc.gpsimd.dma_start(out=t[:], in_=d.rearrange("(a p) -> a p", p=P)[ci][:, None])
                lst.append(t)
        bv_rep = singles.tile([P, C], f32, tag="bv_rep", name="bv_rep")
        nc.gpsimd.dma_start(
            out=bv_rep[:],
            in_=bass.AP(tensor=bv_d.tensor, offset=bv_d.offset,
                        ap=[[0, P]] + list(bv_d.ap)),
        )

        # ---- persistent activations ----
        Q_sb = [singles.tile([P, NN], bf16, tag=f"Q{ci}", name=f"Q{ci}") for ci in range(2)]
        K_sb = [singles.tile([P, M], bf16, tag=f"K{ci}", name=f"K{ci}") for ci in range(2)]
        vt = singles.tile([P, MC, H, D + 1], bf16, tag="vt", name="vt")
        attn = [singles.tile([D, NN], bf16, tag=f"attn{h}", name=f"attn{h}") for h in range(H)]

        nc.vector.memset(vt[:, :, :, D:D + 1], 1.0)

        # ---- inputs (chunked DMAs so the first matmul starts early) ----
        inp = ctx.enter_context(tc.tile_pool(name="inp", bufs=1))
        src_t = [inp.tile([P, M], bf16, tag=f"srcin{ci}", name=f"srcin{ci}")
                 for ci in range(2)]
        q_t = [inp.tile([P, NN], bf16, tag=f"qin{ci}", name=f"qin{ci}")
               for ci in range(2)]
        for c4 in range(8):       # src first (K then VT proj need it first),
            for ci in range(2):   # column-chunked so t=0 lands quickly,
                eng = nc.sync if ci == 0 else nc.scalar  # two DGE queues
                eng.dma_start(out=src_t[ci][:, c4 * 512:(c4 + 1) * 512],
                              in_=src_r[ci][:, c4 * 512:(c4 + 1) * 512])
        for c4 in range(4):
            for ci in range(2):
                nc.gpsimd.dma_start(out=q_t[ci][:, c4 * 512:(c4 + 1) * 512],
                                    in_=q_r[ci][:, c4 * 512:(c4 + 1) * 512])

        # ---- projections (bf16 inputs, fp32 PSUM accumulate) ----
        def proj_one(co, wt, xin, xlen, dst, bias):
            # c_out partitions, sequence on free dim
            if True:
                for t in range(xlen // 512):
                    ps = opool.tile([P, 512], f32, tag="outT", name="ps")
                    for ci in range(2):
                        nc.tensor.matmul(
                            ps[:],
                            wt[ci][:, co * P:(co + 1) * P],
                            xin[ci][:, t * 512:(t + 1) * 512],
                            start=(ci == 0), stop=(ci == 1),
                        )
                    nc.vector.tensor_scalar_add(
                        dst[co][:, t * 512:(t + 1) * 512], ps[:], bias[co])

        def proj_vt_chunk(mc):
            # VT = src.T @ WvT (m partitions, c_out free) + bv, stored as
            # per-head [VT_h | ones] blocks of width D+1
            if True:
                ps = opool.tile([P, 512], f32, tag="outT", name="psv")[:, 0:C]
                for ci in range(2):
                    nc.tensor.matmul(
                        ps[:],
                        src_t[ci][:, mc * P:(mc + 1) * P],
                        wvt[ci][:],
                        start=(ci == 0), stop=(ci == 1),
                    )
                nc.vector.tensor_tensor(
                    vt[:, mc, :, 0:D],
                    ps.rearrange("p (h d) -> p h d", h=H),
                    bv_rep.rearrange("p (h d) -> p h d", h=H),
                    add,
                )

        # ---- attention ----
        LAG = 2  # outT matmuls trail scores by LAG m-chunks so the PE
        #          never sits in its FIFO waiting on ACT's exp

        def attention_tile(h, nt, emit_vt=False):
            ch, off = h // 2, D * (h % 2)
            if True:
                n0 = nt * 1024
                outT = opool.tile([D + 1, 1024], f32, tag="outT", name="outT")
                prs = {}
                for mc in range(MC + LAG):
                    if emit_vt and mc < MC:
                        proj_vt_chunk(mc)
                    if mc < MC:
                        sc = spool.tile([P, 1024], f32, tag="sc", name="sc")
                        for ns in range(2):
                            nc.tensor.matmul(
                                sc[:, ns * 512:(ns + 1) * 512],
                                K_sb[ch][off:off + D, mc * P:(mc + 1) * P],
                                Q_sb[ch][off:off + D,
                                         n0 + ns * 512:n0 + (ns + 1) * 512],
                                start=True, stop=True,
                            )
                        pr = probs_p.tile([P, 1024], bf16, tag="pr", name="pr")
                        nc.scalar.activation(
                            out=pr[:], in_=sc[:], func=Exp, scale=0.125)
                        prs[mc] = pr
                    if mc >= LAG:
                        j = mc - LAG
                        pr_j = prs.pop(j)
                        for ns in range(2):
                            nc.tensor.matmul(
                                outT[:, ns * 512:(ns + 1) * 512],
                                vt[:, j, h, :],
                                pr_j[:, ns * 512:(ns + 1) * 512],
                                start=(j == 0), stop=(j == MC - 1),
                            )
                # row D of outT = softmax denominators (ones column of vt).
                # Plain copy handles the PSUM p64 -> SBUF p0 partition shift;
                # the custom-DVE approx reciprocal does not, so keep it on
                # partition-aligned SBUF data.
                den = small_p.tile([1, 1024], f32, tag="den", name="den")
                nc.vector.tensor_copy(out=den[0:1, :], in_=outT[D:D + 1, :])
                rec = small_p.tile([1, 1024], f32, tag="rec", name="rec")
                scr = small_p.tile([1, 1024], f32, tag="scr", name="scr")
                nc.vector.reciprocal_approx_accurate(
                    out=rec[0:1, :], in_=den[0:1, :], scratch=scr[0:1, :])
                # partition-broadcast via DRAM bounce (PSUM is DMA-opaque;
                # gpsimd traffic locks the DVE-shared SBUF port, so use DMA)
                dscr = dram_p.tile([1, 1024], f32, name="dscr")
                nc.sync.dma_start(out=dscr[:], in_=rec[0:1, :])
                rrep = small_p.tile([D, 1024], f32, tag="rrep", name="rrep")
                nc.sync.dma_start(
                    out=rrep[:],
                    in_=bass.AP(tensor=dscr.tensor, offset=dscr.offset,
                                ap=[[0, D]] + list(dscr.ap)[1:]))
                nc.vector.tensor_tensor(
                    attn[h][:, n0:n0 + 1024],
                    outT[0:D, :],
                    rrep[:],
                    mult,
                )

        def merge_nt(nt):
            for co in range(2):
                for t in range(2 * nt, 2 * nt + 2):
                    ps = opool.tile([P, 512], f32, tag="outT", name="psm")
                    for h in range(H):
                        nc.tensor.matmul(
                            ps[:],
                            wm_h[h][:, co * P:(co + 1) * P],
                            attn[h][:, t * 512:(t + 1) * 512],
                            start=(h == 0), stop=(h == H - 1),
                        )
                    ot = ostage.tile([P, 512], f32, tag="ot", name="ot")
                    nc.vector.tensor_scalar_add(ot[:], ps[:], bm_t[co])
                    nc.sync.dma_start(out=y_r[co, :, t * 512:(t + 1) * 512],
                                      in_=ot[:])

        proj_one(0, wkt, src_t, M, K_sb, bk_t)
        proj_one(0, wqt, q_t, NN, Q_sb, bq_t)
        attention_tile(0, 0, emit_vt=True)
        attention_tile(0, 1)
        proj_one(1, wkt, src_t, M, K_sb, bk_t)
        proj_one(1, wqt, q_t, NN, Q_sb, bq_t)
        attention_tile(1, 0)
        attention_tile(1, 1)
        attention_tile(2, 0)
        attention_tile(2, 1)
        attention_tile(3, 0)
        attention_tile(3, 1)
        merge_nt(0)
        merge_nt(1)

    nc.compile()
    return nc


def _get_nc():
    if "nc" not in _STATE:
        _STATE["nc"] = _build()
    return _STATE["nc"]


def kernel(query, source, Wq, bq, Wk, bk, Wv, bv, Wm, bm):
    import ml_dtypes
    from concourse.bass_utils import run_bass_kernel_spmd

    bf16 = ml_dtypes.bfloat16
    query = np.asarray(query, np.float32).astype(bf16)
    source = np.asarray(source, np.float32).astype(bf16)
    wqT = np.ascontiguousarray(np.asarray(Wq, np.float32).T).astype(bf16)
    wkT = np.ascontiguousarray(np.asarray(Wk, np.float32).T).astype(bf16)
    wvT = np.ascontiguousarray(np.asarray(Wv, np.float32).T).astype(bf16)
    wmT = np.ascontiguousarray(np.asarray(Wm, np.float32).T).astype(bf16)
    bq = np.asarray(bq, np.float32)
    bk = np.asarray(bk, np.float32)
    bv = np.asarray(bv, np.float32)
    bm = np.asarray(bm, np.float32)

    nc = _get_nc()

    in_maps = []
    for c in range(N_CORES):
        b, nh = c // 2, c % 2
        in_maps.append({
            "q": np.ascontiguousarray(query[b, :, nh * NN:(nh + 1) * NN]),
            "src": np.ascontiguousarray(source[b]),
            "wqT": wqT, "wkT": wkT, "wvT": wvT, "wmT": wmT,
            "bq": bq, "bk": bk, "bv": bv, "bm": bm,
        })

    trace = os.environ.get("KERNEL_TRACE") == "1"
    res = run_bass_kernel_spmd(
        nc, in_maps, core_ids=list(range(N_CORES)), trace=trace)
    _STATE["last_result"] = res
    if trace and res.exec_time_ns is not None:
        print(f"HW exec time: {res.exec_time_ns} ns")

    out = np.empty((B, C, N), np.float32)
    for c in range(N_CORES):
        b, nh = c // 2, c % 2
        out[b, :, nh * NN:(nh + 1) * NN] = res.results[c]["y"]
    return out
